# revision 1
# baseline (speedup 1.0000x reference)
"""Bass/Trainium2 kernel for nn_MultiHeadAttention (B=4, S=2048, E=512, H=8, dk=dv=8).

Sharding: 8 cores = (batch b, head-half hh).  Core 2b+hh computes causal
attention for batch b over heads [4hh, 4hh+4) for all 2048 queries, applies
its half of the output projection, and returns a partial output transposed
[E, S].  Host sums the two partials per batch, transposes, and adds bo.

Device layout notes:
  - Host feeds query/key/value TRANSPOSED ([E, S]) so projections can use
    them directly as matmul operands (contraction dim on partitions).
  - Projection weights are host-packed into "spread" layouts so projection
    outputs land at partitions {32h+d} (Q/K, row-tiling-ready) or columns
    {9h+d} (V, with a ones column per head at 9h+8 for the softmax
    denominator).
  - Scores are computed transposed ([t, q]) so exp(scores) tiles feed the
    A@V matmul as the moving operand with V as the (cheap) stationary one.
  - 4 heads run concurrently on the PE array: scores via row-tiling
    (tile_position=(32h, 0), K=8) and A@V via col-tiling
    (tile_position=(0, 32h), M=9).
  - Softmax normalization is folded to the end: A@V accumulates
    unnormalized o^T plus a denominator row per head; reciprocal +
    partition-broadcast + one multiply normalize before the out-proj.
"""

import math

import numpy as np

B, S, E, H = 4, 2048, 512, 8
DK_H = DV_H = 8
NCORES = 8
HPC = H // 2  # heads per core = 4
SCALE = 1.0 / math.sqrt(DK_H)
NEG = -1.0e30
NQC = S // 512  # q chunks of 512
NTB = S // 128  # t blocks of 128
ECH = E // 128  # e chunks of 128

_cache: dict = {}


def _apply_tile_patch():
    """walrus in this image allows only one sync-wait per Drain; split the
    TileContext tail drain's waits across a chain of drains."""
    import concourse.mybir as mybir
    from concourse import tile
    from concourse.vector_clock import ScopedClock

    if getattr(tile.TileContext._drain_and_barrier, "_split_patch", False):
        return

    def _drain_and_barrier_split(self, tick_clock, wait_clock):
        drain_inst = self.nc.sync.drain()
        wait_clock.add_sem_waits(
            drain_inst.ins, ScopedClock({None: tick_clock.global_clock})
        )
        si = drain_inst.ins.sync_info
        if si is not None and si.on_wait and len(si.on_wait) > 1:
            waits = list(si.on_wait)
            si.on_wait = waits[:1]
            for entry in waits[1:]:
                extra = self.nc.sync.drain()
                extra.ins.sync_info = mybir.SyncInfo(on_wait=[entry], on_update=[])
        self.nc.all_engine_barrier()
        assert self.sems is not None
        popped = self.nc._tile_sem_poison_stack.pop()
        assert popped is self._sem_poison
        self.nc.clear_and_free_semaphores(list(self.sems.allocated().values()))
        self.nc.all_engine_barrier()

    _drain_and_barrier_split._split_patch = True
    tile.TileContext._drain_and_barrier = _drain_and_barrier_split


def _split_multi_waits(nc):
    """walrus in this image allows only one sync-wait per instruction;
    move excess waits onto single-wait NOPs inserted just before."""
    import concourse.mybir as mybir

    for blk in nc.m.functions[0].blocks:
        out = []
        for inst in blk.instructions:
            si = getattr(inst, "sync_info", None)
            if si is not None and si.on_wait and len(si.on_wait) > 1:
                waits = list(si.on_wait)
                for i, entry in enumerate(waits[:-1]):
                    out.append(
                        mybir.InstNoOp(
                            name=f"{inst.name}_w{i}",
                            engine=inst.engine,
                            ins=[],
                            outs=[],
                            bass_nofuse=True,
                            sync_info=mybir.SyncInfo(
                                on_wait=[entry], on_update=[]
                            ),
                        )
                    )
                si.on_wait = waits[-1:]
            out.append(inst)
        blk.instructions = out


def _build():
    import concourse.bass as bass  # noqa: F401
    import concourse.mybir as mybir
    from concourse import tile

    _apply_tile_patch()
    f32 = mybir.dt.float32
    Exp = mybir.ActivationFunctionType.Exp

    import concourse.bass as bassmod

    f32r = mybir.dt.float32r

    def rr(ap):
        return ap.bitcast(f32r)

    nc = bassmod.Bass()
    qT = nc.declare_dram_parameter("qT", [E, S], f32r, isOutput=False)
    kT = nc.declare_dram_parameter("kT", [E, S], f32r, isOutput=False)
    vT = nc.declare_dram_parameter("vT", [E, S], f32r, isOutput=False)
    wq = nc.declare_dram_parameter("wq", [E, 128], f32r, isOutput=False)
    wk = nc.declare_dram_parameter("wk", [E, 128], f32r, isOutput=False)
    wv = nc.declare_dram_parameter("wv", [E, HPC * 9], f32r, isOutput=False)
    wo = nc.declare_dram_parameter("wo", [128, E], f32r, isOutput=False)
    msk = nc.declare_dram_parameter("msk", [128, 2 * 128], f32, isOutput=False)
    out = nc.declare_dram_parameter("out", [E, S], f32, isOutput=True)

    with tile.TileContext(nc) as tc:
        with (
            tc.tile_pool(name="singles", bufs=1) as singles,
            tc.tile_pool(name="loads", bufs=6) as loads,
            tc.tile_pool(name="abuf", bufs=3) as abuf,
            tc.tile_pool(name="outs", bufs=3) as outs,
            tc.tile_pool(name="ps_sc", bufs=3, space="PSUM") as ps_sc,
            tc.tile_pool(name="ps_av", bufs=1, space="PSUM") as ps_av,
            tc.tile_pool(name="ps_misc", bufs=1, space="PSUM") as ps_misc,
        ):
            # ---- resident tensors -------------------------------------
            wq_sb = singles.tile([128, ECH, 128], f32r, tag="wq")
            wk_sb = singles.tile([128, ECH, 128], f32r, tag="wk")
            wv_sb = singles.tile([128, ECH, HPC * 9], f32r, tag="wv")
            wo_sb = singles.tile([128, ECH, 128], f32r, tag="wo")
            msk_sb = singles.tile([128, 2, 128], f32, tag="msk")
            nc.sync.dma_start(out=wq_sb, in_=wq.rearrange("(c p) m -> p c m", p=128))
            nc.sync.dma_start(out=wk_sb, in_=wk.rearrange("(c p) m -> p c m", p=128))
            nc.sync.dma_start(out=wv_sb, in_=wv.rearrange("(c p) m -> p c m", p=128))
            nc.sync.dma_start(out=wo_sb, in_=wo.rearrange("p (c m) -> p c m", c=ECH))
            nc.sync.dma_start(out=msk_sb, in_=msk.rearrange("p (g n) -> p g n", g=2))

            KTs = singles.tile([128, S], f32r, tag="KTs")
            QTs = singles.tile([128, S], f32r, tag="QTs")
            Vsb = singles.tile([128, NTB, HPC, 9], f32, tag="Vsb")
            onorm = singles.tile([128, S], f32r, tag="onorm")
            recip = singles.tile([128, S], f32, tag="recip")
            recip_rep = singles.tile([128, S], f32, tag="recip_rep")

            ones9 = singles.tile([128, 9], f32, tag="ones9")
            nc.vector.memset(ones9, 1.0)

            # zero backgrounds (stale SBUF must not reach the PE as NaN)
            nc.vector.memset(onorm.bitcast(f32), 0.0)
            # ones columns for the denominator accumulation
            nc.vector.memset(Vsb[:, :, :, 0:1], 1.0)

            # ---- phase A: projections ---------------------------------
            for t in range(NQC):  # 512-col chunks of the sequence
                cs = slice(t * 512, (t + 1) * 512)
                k_tiles = []
                v_tiles = []
                q_tiles = []
                for e in range(ECH):
                    ktile = loads.tile([128, 512], f32r, tag="ld")
                    nc.sync.dma_start(
                        out=ktile, in_=kT[e * 128:(e + 1) * 128, cs]
                    )
                    k_tiles.append(ktile)
                kt_ps = ps_misc.tile([128, 512], f32, tag="ps")
                for e in range(ECH):
                    nc.tensor.matmul(
                        kt_ps, wk_sb[:, e, :], k_tiles[e][:, :],
                        start=(e == 0), stop=(e == ECH - 1),
                    )
                nc.vector.tensor_copy(KTs[:, cs], kt_ps)

                for e in range(ECH):
                    vtile = loads.tile([128, 512], f32r, tag="ld")
                    nc.sync.dma_start(
                        out=vtile, in_=vT[e * 128:(e + 1) * 128, cs]
                    )
                    v_tiles.append(vtile)
                for tb in range(4):  # 128-row t blocks within the chunk
                    v_ps = ps_misc.tile([128, HPC * 9], f32, tag="ps")
                    for e in range(ECH):
                        nc.tensor.matmul(
                            v_ps,
                            v_tiles[e][:, tb * 128:(tb + 1) * 128],
                            wv_sb[:, e, :],
                            start=(e == 0), stop=(e == ECH - 1),
                        )
                    dst = Vsb[:, 4 * t + tb, :, 1:9]
                    src = v_ps.rearrange("p (h n) -> p h n", n=9)[:, :, 1:9]
                    nc.vector.tensor_copy(dst, src)

                for e in range(ECH):
                    qtile = loads.tile([128, 512], f32r, tag="ld")
                    nc.sync.dma_start(
                        out=qtile, in_=qT[e * 128:(e + 1) * 128, cs]
                    )
                    q_tiles.append(qtile)
                qt_ps = ps_misc.tile([128, 512], f32, tag="ps")
                for e in range(ECH):
                    nc.tensor.matmul(
                        qt_ps, wq_sb[:, e, :], q_tiles[e][:, :],
                        start=(e == 0), stop=(e == ECH - 1),
                    )
                nc.vector.tensor_copy(QTs[:, cs], qt_ps)

            # ---- phase B: attention -----------------------------------
            for c in range(NQC):
                cs = slice(c * 512, (c + 1) * 512)
                av = ps_av.tile([128, 512], f32, tag="av")
                ntb = 4 * (c + 1)
                for tb in range(ntb):
                    d = 128 * tb - 512 * c  # diagonal offset within the chunk
                    scs = [
                        ps_sc.tile([128, 2, 512], f32, tag="sc", name=f"sc{c}_{tb}_0"),
                        ps_sc.tile([128, 2, 512], f32, tag="sc", name=f"sc{c}_{tb}_1"),
                    ]
                    ats = [
                        abuf.tile([128, 2, 512], f32, tag="a", name=f"a{c}_{tb}_0"),
                        abuf.tile([128, 2, 512], f32, tag="a", name=f"a{c}_{tb}_1"),
                    ]
                    for h in range(HPC):
                        g, j = divmod(h, 2)
                        nc.tensor.matmul(
                            scs[g][:, j, :],
                            KTs[32 * h:32 * h + 8, tb * 128:(tb + 1) * 128],
                            QTs[32 * h:32 * h + 8, cs],
                            start=True, stop=True,
                            tile_position=(32 * h, 0),
                        )
                    vstart = max(d, 0)
                    for g in range(2):
                        if d >= 0:
                            nc.vector.tensor_add(
                                scs[g][:, :, d:d + 128],
                                scs[g][:, :, d:d + 128],
                                msk_sb,
                            )
                        nc.scalar.activation(
                            ats[g][:, :, vstart:512], scs[g][:, :, vstart:512],
                            Exp, scale=SCALE,
                        )
                    for h in range(HPC):
                        g, j = divmod(h, 2)
                        nc.tensor.matmul(
                            av[32 * h:32 * h + 9, vstart:512],
                            Vsb[:, tb, h, :],
                            ats[g][:, j, vstart:512],
                            start=(tb == 0), stop=(tb == ntb - 1),
                            tile_position=(0, 32 * h),
                        )
                # normalize: reciprocal of the denom row, replicate it across
                # the head's 9 partitions via a K=1 rank-1 matmul, multiply.
                for h in range(HPC):
                    nc.vector.reciprocal(
                        recip[32 * h:32 * h + 1, cs],
                        av[32 * h:32 * h + 1, :],
                    )
                rep_ps = ps_misc.tile([128, 512], f32, tag="ps", name=f"rep{c}")
                for h in range(HPC):
                    nc.tensor.matmul(
                        rep_ps[32 * h:32 * h + 9, :],
                        ones9[32 * h:32 * h + 1, :],
                        recip[32 * h:32 * h + 1, cs],
                        start=True, stop=True,
                        tile_position=(32 * h, 32 * h),
                    )
                for h in range(HPC):
                    nc.vector.tensor_copy(
                        recip_rep[32 * h:32 * h + 9, cs],
                        rep_ps[32 * h:32 * h + 9, :],
                    )
                for h in range(HPC):
                    nc.vector.tensor_mul(
                        onorm[32 * h:32 * h + 9, cs],
                        av[32 * h:32 * h + 9, :],
                        recip_rep[32 * h:32 * h + 9, cs],
                    )
                # out projection for this q chunk
                for e in range(ECH):
                    f_ps = ps_misc.tile([128, 512], f32, tag="ps")
                    nc.tensor.matmul(
                        f_ps, wo_sb[:, e, :], onorm[:, cs],
                        start=True, stop=True,
                    )
                    fsb = outs.tile([128, 512], f32, tag="f")
                    nc.vector.tensor_copy(fsb, f_ps)
                    nc.sync.dma_start(
                        out=out[e * 128:(e + 1) * 128, cs], in_=fsb
                    )
    _split_multi_waits(nc)
    return nc


def _prep_inputs(query, key, value, Wq, Wk, Wv, Wo):
    """Build the 8 per-core input maps (host-side sharding/layout)."""
    qTs = [np.ascontiguousarray(query[b].T) for b in range(B)]
    kTs = [np.ascontiguousarray(key[b].T) for b in range(B)]
    vTs = [np.ascontiguousarray(value[b].T) for b in range(B)]

    mask = np.where(
        np.arange(128)[:, None] <= np.arange(128)[None, :], 0.0, NEG
    ).astype(np.float32)
    msk2 = np.ascontiguousarray(np.tile(mask, (1, 2)))

    in_maps = []
    for core in range(NCORES):
        b, hh = divmod(core, 2)
        wq_p = np.zeros((E, 128), np.float32)
        wk_p = np.zeros((E, 128), np.float32)
        wv_p = np.zeros((E, HPC * 9), np.float32)
        wo_p = np.zeros((128, E), np.float32)
        for h in range(HPC):
            g = 4 * hh + h
            wq_p[:, 32 * h:32 * h + 8] = Wq[g]
            wk_p[:, 32 * h:32 * h + 8] = Wk[g]
            wv_p[:, 9 * h + 1:9 * h + 9] = Wv[g]
            wo_p[32 * h + 1:32 * h + 9, :] = Wo[8 * g:8 * g + 8, :]
        in_maps.append(
            {
                "qT": qTs[b], "kT": kTs[b], "vT": vTs[b],
                "wq": wq_p, "wk": wk_p, "wv": wv_p, "wo": wo_p,
                "msk": msk2,
            }
        )
    return in_maps


def _reference_numpy(query, key, value, padding_mask, decoder_mask,
                     Wq, Wk, Wv, Wo, bo):
    """Fallback (non-default masks): plain numpy replica of the reference."""
    q = np.einsum("bse,hed->bhsd", query, Wq)
    k = np.einsum("bse,hed->bhsd", key, Wk)
    v = np.einsum("bse,hed->bhsd", value, Wv)
    s = np.einsum("bhsd,bhtd->bhst", q, k)
    if decoder_mask:
        tril = np.tril(s)
        s = np.where(tril == 0.0, -np.inf, s)
    s = np.where(padding_mask[:, None, :, :], s, -np.inf)
    s = s / np.sqrt(np.float32(DK_H))
    m = np.max(s, axis=-1, keepdims=True)
    e = np.exp(s - m)
    a = e / np.sum(e, axis=-1, keepdims=True)
    o = np.einsum("bhst,bhtd->bhsd", a, v)
    o = o.transpose(0, 2, 1, 3).reshape(o.shape[0], o.shape[2], H * DV_H)
    return (o @ Wo + bo).astype(np.float32)


def kernel(query, key, value, padding_mask, decoder_mask, Wq, Wk, Wv, Wo, bo,
           **run_kwargs):
    query = np.asarray(query, np.float32)
    key = np.asarray(key, np.float32)
    value = np.asarray(value, np.float32)
    Wq = np.asarray(Wq, np.float32)
    Wk = np.asarray(Wk, np.float32)
    Wv = np.asarray(Wv, np.float32)
    Wo = np.asarray(Wo, np.float32)
    bo = np.asarray(bo, np.float32)
    pm = np.asarray(padding_mask)
    dm = int(np.asarray(decoder_mask))

    if not bool(pm.all()) or not dm:
        return _reference_numpy(
            query, key, value, pm.astype(bool), dm, Wq, Wk, Wv, Wo, bo
        )

    from concourse.bass_utils import run_bass_kernel_spmd

    if "nc" not in _cache:
        _cache["nc"] = _build()
    nc = _cache["nc"]

    in_maps = _prep_inputs(query, key, value, Wq, Wk, Wv, Wo)
    res = run_bass_kernel_spmd(nc, in_maps, list(range(NCORES)), **run_kwargs)

    outp = np.empty((B, S, E), np.float32)
    for b in range(B):
        fT = res.results[2 * b]["out"] + res.results[2 * b + 1]["out"]
        outp[b] = fT.T + bo
    if run_kwargs:
        kernel.last_result = res
    return outp



# revision 5
# speedup vs baseline: 1.6460x; 1.6460x over previous
"""Bass/Trainium2 kernel for nn_MultiHeadAttention (B=4, S=2048, E=512, H=8, dk=dv=8).

Sharding: 8 cores = (batch b, head-half hh).  Core 2b+hh computes causal
attention for batch b over heads [4hh, 4hh+4) for all 2048 queries, applies
its half of the output projection, and returns a partial output transposed
[E, S] in bf16.  Host sums the two partials per batch, transposes, adds bo.

Device layout notes (v2 — bf16 datapath):
  - Everything the PE touches is bf16 (fp32-HIGH matmuls are ~2.5x slower
    and block fast-weight-load).  PSUM accumulation stays f32.
  - Host feeds query/key/value TRANSPOSED ([E, S]) bf16; q/k/v live whole
    in SBUF (16 KiB/partition each), loaded with per-chunk DMAs.
  - Projection weights host-packed "spread": Q/K heads at partition cols
    {32h..32h+8}; V at cols {9h+1..9h+9} of a 36-wide block (col 9h+0
    feeds the softmax denominator via a ones column); Wo rows at
    {32h+1..32h+9}.
  - Scores are computed transposed ([t, q]): 4 heads concurrently via PE
    row-tiling (tile_position=(32h,0), K=8), one [128,4,512] PSUM tile
    (4 banks), ONE 2048-wide ACT exp per (chunk, t-block) writing bf16.
  - The causal mask of diagonal blocks is ADDED ON THE PE (stationary
    mask^T x identity accumulated into the scores psum) so no vector-op
    sits between matmul and exp.
  - A@V via col-tiling (tile_position=(0,32h)) with a [128 t, 32] bf16
    stationary (8 V dims + ones + 23 zero cols) so the whole [128,512]
    accumulator psum is matmul-written (no uninitialized reads).
  - Softmax normalization folded to the end of each chunk: one full-width
    DVE reciprocal_approx_fast on the accumulator, a rank-1 PE matmul
    replicates each head's denominator row across its 32-partition group,
    one full-width DVE multiply produces the bf16 out-proj operand.
"""

import math

import numpy as np

B, S, E, H = 4, 2048, 512, 8
DK_H = DV_H = 8
NCORES = 8
HPC = H // 2  # heads per core = 4
SCALE = 1.0 / math.sqrt(DK_H)
NEG = -1.0e30
NQC = S // 512  # q chunks of 512
NTB = S // 128  # t blocks of 128
ECH = E // 128  # e chunks of 128

_cache: dict = {}


def _apply_tile_patch():
    """walrus in this image allows only one sync-wait per Drain; split the
    TileContext tail drain's waits across a chain of drains."""
    import concourse.mybir as mybir
    from concourse import tile
    from concourse.vector_clock import ScopedClock

    if getattr(tile.TileContext._drain_and_barrier, "_split_patch", False):
        return

    def _drain_and_barrier_split(self, tick_clock, wait_clock):
        drain_inst = self.nc.sync.drain()
        wait_clock.add_sem_waits(
            drain_inst.ins, ScopedClock({None: tick_clock.global_clock})
        )
        si = drain_inst.ins.sync_info
        if si is not None and si.on_wait and len(si.on_wait) > 1:
            waits = list(si.on_wait)
            si.on_wait = waits[:1]
            for entry in waits[1:]:
                extra = self.nc.sync.drain()
                extra.ins.sync_info = mybir.SyncInfo(on_wait=[entry], on_update=[])
        self.nc.all_engine_barrier()
        assert self.sems is not None
        popped = self.nc._tile_sem_poison_stack.pop()
        assert popped is self._sem_poison
        self.nc.clear_and_free_semaphores(list(self.sems.allocated().values()))
        self.nc.all_engine_barrier()

    _drain_and_barrier_split._split_patch = True
    tile.TileContext._drain_and_barrier = _drain_and_barrier_split


def _split_multi_waits(nc):
    """walrus in this image allows only one sync-wait per instruction;
    move excess waits onto single-wait NOPs inserted just before."""
    import concourse.mybir as mybir

    for blk in nc.m.functions[0].blocks:
        out = []
        for inst in blk.instructions:
            si = getattr(inst, "sync_info", None)
            if si is not None and si.on_wait and len(si.on_wait) > 1:
                waits = list(si.on_wait)
                for i, entry in enumerate(waits[:-1]):
                    out.append(
                        mybir.InstNoOp(
                            name=f"{inst.name}_w{i}",
                            engine=inst.engine,
                            ins=[],
                            outs=[],
                            bass_nofuse=True,
                            sync_info=mybir.SyncInfo(
                                on_wait=[entry], on_update=[]
                            ),
                        )
                    )
                si.on_wait = waits[-1:]
            out.append(inst)
        blk.instructions = out


def _build():
    import concourse.bass as bassmod
    import concourse.mybir as mybir
    from concourse import tile

    _apply_tile_patch()
    f32 = mybir.dt.float32
    bf16 = mybir.dt.bfloat16
    Exp = mybir.ActivationFunctionType.Exp

    nc = bassmod.Bass()
    qT = nc.declare_dram_parameter("qT", [E, S], bf16, isOutput=False)
    kT = nc.declare_dram_parameter("kT", [E, S], bf16, isOutput=False)
    vT = nc.declare_dram_parameter("vT", [E, S], bf16, isOutput=False)
    wq = nc.declare_dram_parameter("wq", [E, 128], bf16, isOutput=False)
    wk = nc.declare_dram_parameter("wk", [E, 128], bf16, isOutput=False)
    wv = nc.declare_dram_parameter("wv", [E, HPC * 9], bf16, isOutput=False)
    wo = nc.declare_dram_parameter("wo", [128, E], bf16, isOutput=False)
    mskT = nc.declare_dram_parameter("mskT", [128, 128], bf16, isOutput=False)
    ident = nc.declare_dram_parameter("ident", [128, 128], bf16, isOutput=False)
    out = nc.declare_dram_parameter("out", [E, S], bf16, isOutput=True)

    with tile.TileContext(nc) as tc:
        with (
            tc.tile_pool(name="singles", bufs=1) as singles,
            tc.tile_pool(name="ats", bufs=3) as atpool,
            tc.tile_pool(name="onorm", bufs=2) as onpool,
            tc.tile_pool(name="outs", bufs=2) as outs,
            tc.tile_pool(name="rc", bufs=2) as rcpool,
            tc.tile_pool(name="ps_sc", bufs=1, space="PSUM") as ps_sc,
            tc.tile_pool(name="ps_av", bufs=2, space="PSUM") as ps_av,
            tc.tile_pool(name="ps_misc", bufs=2, space="PSUM") as ps_misc,
        ):
            # ---- resident tensors -------------------------------------
            wq_sb = singles.tile([128, ECH, 128], bf16, tag="wq")
            wk_sb = singles.tile([128, ECH, 128], bf16, tag="wk")
            wv_sb = singles.tile([128, ECH, HPC * 9], bf16, tag="wv")
            wo_sb = singles.tile([128, ECH, 128], bf16, tag="wo")
            msk_sb = singles.tile([128, 128], bf16, tag="msk")
            id_sb = singles.tile([128, 128], bf16, tag="id")
            nc.sync.dma_start(out=wq_sb, in_=wq.rearrange("(c p) m -> p c m", p=128))
            nc.sync.dma_start(out=wk_sb, in_=wk.rearrange("(c p) m -> p c m", p=128))
            nc.sync.dma_start(out=wv_sb, in_=wv.rearrange("(c p) m -> p c m", p=128))
            nc.sync.dma_start(out=wo_sb, in_=wo.rearrange("p (c m) -> p c m", c=ECH))
            nc.sync.dma_start(out=msk_sb, in_=mskT[:, :])
            nc.sync.dma_start(out=id_sb, in_=ident[:, :])

            q_sb = singles.tile([128, ECH, S], bf16, tag="q")
            k_sb = singles.tile([128, ECH, S], bf16, tag="k")
            v_sb = singles.tile([128, ECH, S], bf16, tag="v")
            kTr = kT.rearrange("(c p) m -> p c m", p=128)
            vTr = vT.rearrange("(c p) m -> p c m", p=128)
            qTr = qT.rearrange("(c p) m -> p c m", p=128)
            for t in range(NQC):
                cs = slice(t * 512, (t + 1) * 512)
                nc.sync.dma_start(out=k_sb[:, :, cs], in_=kTr[:, :, cs])
                nc.sync.dma_start(out=v_sb[:, :, cs], in_=vTr[:, :, cs])
                nc.sync.dma_start(out=q_sb[:, :, cs], in_=qTr[:, :, cs])

            KTs = singles.tile([128, S], bf16, tag="KTs")
            QTs = singles.tile([128, S], bf16, tag="QTs")
            Vsb = singles.tile([128, NTB, HPC, 32], bf16, tag="Vsb")
            ones32 = singles.tile([128, 32], f32, tag="ones32")
            nc.vector.memset(ones32, 1.0)
            # V stationary: col 0 = ones (denominator), 1..8 = V dims,
            # 9..31 = zeros (pads the A@V psum write to the full group).
            nc.vector.memset(Vsb[:, :, :, 0:1], 1.0)
            nc.vector.memset(Vsb[:, :, :, 9:32], 0.0)

            # ---- phase A: projections ---------------------------------
            for t in range(NQC):  # 512-col chunks of the sequence
                cs = slice(t * 512, (t + 1) * 512)
                kt_ps = ps_misc.tile([128, 512], f32, tag="ps")
                for e in range(ECH):
                    nc.tensor.matmul(
                        kt_ps, wk_sb[:, e, :], k_sb[:, e, cs],
                        start=(e == 0), stop=(e == ECH - 1),
                    )
                nc.vector.tensor_copy(KTs[:, cs], kt_ps)

                for tb in range(4):  # 128-row t blocks within the chunk
                    bs = slice(t * 512 + tb * 128, t * 512 + (tb + 1) * 128)
                    v_ps = ps_misc.tile([128, HPC * 9], f32, tag="ps")
                    for e in range(ECH):
                        nc.tensor.matmul(
                            v_ps,
                            v_sb[:, e, bs],
                            wv_sb[:, e, :],
                            start=(e == 0), stop=(e == ECH - 1),
                        )
                    dst = Vsb[:, 4 * t + tb, :, 1:9]
                    src = v_ps.rearrange("p (h n) -> p h n", n=9)[:, :, 1:9]
                    nc.vector.tensor_copy(dst, src)

                qt_ps = ps_misc.tile([128, 512], f32, tag="ps")
                for e in range(ECH):
                    nc.tensor.matmul(
                        qt_ps, wq_sb[:, e, :], q_sb[:, e, cs],
                        start=(e == 0), stop=(e == ECH - 1),
                    )
                nc.vector.tensor_copy(QTs[:, cs], qt_ps)

            # ---- phase B: attention -----------------------------------
            outr = out.rearrange("(c p) m -> p c m", p=128)
            for c in range(NQC):
                cs = slice(c * 512, (c + 1) * 512)
                av = ps_av.tile([128, 512], f32, tag="av")
                ntb = 4 * (c + 1)
                for tb in range(ntb):
                    d = 128 * tb - 512 * c  # diagonal offset within the chunk
                    vstart = max(d, 0)
                    sc = ps_sc.tile([128, HPC, 512], f32, tag="sc",
                                    name=f"sc{c}_{tb}")
                    at = atpool.tile([128, HPC, 512], bf16, tag="a",
                                     name=f"a{c}_{tb}")
                    for h in range(HPC):
                        nc.tensor.matmul(
                            sc[:, h, :],
                            KTs[32 * h:32 * h + 8, tb * 128:(tb + 1) * 128],
                            QTs[32 * h:32 * h + 8, cs],
                            start=True, stop=(d < 0),
                            tile_position=(32 * h, 0),
                        )
                    if d >= 0:
                        # causal mask of the diagonal 128-col region, added
                        # on the PE: msk^T (stationary) @ I (moving).
                        for h in range(HPC):
                            nc.tensor.matmul(
                                sc[:, h, d:d + 128],
                                msk_sb,
                                id_sb,
                                start=False, stop=True,
                            )
                    nc.scalar.activation(
                        at[:, :, vstart:512], sc[:, :, vstart:512],
                        Exp, scale=SCALE,
                    )
                    for h in range(HPC):
                        nc.tensor.matmul(
                            av[32 * h:32 * h + 32, vstart:512],
                            Vsb[:, tb, h, :],
                            at[:, h, vstart:512],
                            start=(tb == 0), stop=(tb == ntb - 1),
                            tile_position=(0, 32 * h),
                        )
                # normalize: full-width approx reciprocal of the psum (the
                # denominator rows among it), replicate each head's denom
                # row across its 32-partition group via a K=1 rank-1
                # matmul, then one full-width multiply -> bf16 operand.
                recipf = rcpool.tile([128, 512], f32, tag="rc", name=f"rc{c}")
                nc.vector.reciprocal(recipf, av)
                rep_ps = ps_misc.tile([128, 512], f32, tag="ps", name=f"rep{c}")
                for h in range(HPC):
                    nc.tensor.matmul(
                        rep_ps[32 * h:32 * h + 32, :],
                        ones32[32 * h:32 * h + 1, :],
                        recipf[32 * h:32 * h + 1, :],
                        start=True, stop=True,
                        tile_position=(32 * h, 32 * h),
                    )
                rep_sb = rcpool.tile([128, 512], f32, tag="rs", name=f"rs{c}")
                nc.vector.tensor_copy(rep_sb, rep_ps)
                onorm = onpool.tile([128, 512], bf16, tag="on", name=f"on{c}")
                nc.vector.tensor_mul(onorm, av, rep_sb)
                # out projection for this q chunk
                fsb = outs.tile([128, ECH, 512], bf16, tag="f", name=f"f{c}")
                for e in range(ECH):
                    f_ps = ps_misc.tile([128, 512], f32, tag="ps")
                    nc.tensor.matmul(
                        f_ps, wo_sb[:, e, :], onorm,
                        start=True, stop=True,
                    )
                    nc.vector.tensor_copy(fsb[:, e, :], f_ps)
                nc.sync.dma_start(out=outr[:, :, cs], in_=fsb)
    _split_multi_waits(nc)
    return nc


def _prep_inputs(query, key, value, Wq, Wk, Wv, Wo):
    """Build the 8 per-core input maps (host-side sharding/layout)."""
    import ml_dtypes

    bf = ml_dtypes.bfloat16
    qTs = [np.ascontiguousarray(query[b].T).astype(bf) for b in range(B)]
    kTs = [np.ascontiguousarray(key[b].T).astype(bf) for b in range(B)]
    vTs = [np.ascontiguousarray(value[b].T).astype(bf) for b in range(B)]

    # mskT[k, m] = 0 if m <= k else NEG  (== msk.T with msk[t, q] causal)
    mskT = np.where(
        np.arange(128)[None, :] <= np.arange(128)[:, None], 0.0, NEG
    ).astype(np.float32).astype(bf)
    mskT = np.ascontiguousarray(mskT)
    ident = np.eye(128, dtype=np.float32).astype(bf)

    in_maps = []
    for core in range(NCORES):
        b, hh = divmod(core, 2)
        wq_p = np.zeros((E, 128), np.float32)
        wk_p = np.zeros((E, 128), np.float32)
        wv_p = np.zeros((E, HPC * 9), np.float32)
        wo_p = np.zeros((128, E), np.float32)
        for h in range(HPC):
            g = 4 * hh + h
            wq_p[:, 32 * h:32 * h + 8] = Wq[g]
            wk_p[:, 32 * h:32 * h + 8] = Wk[g]
            wv_p[:, 9 * h + 1:9 * h + 9] = Wv[g]
            wo_p[32 * h + 1:32 * h + 9, :] = Wo[8 * g:8 * g + 8, :]
        in_maps.append(
            {
                "qT": qTs[b], "kT": kTs[b], "vT": vTs[b],
                "wq": wq_p.astype(bf), "wk": wk_p.astype(bf),
                "wv": wv_p.astype(bf), "wo": wo_p.astype(bf),
                "mskT": mskT, "ident": ident,
            }
        )
    return in_maps


def _reference_numpy(query, key, value, padding_mask, decoder_mask,
                     Wq, Wk, Wv, Wo, bo):
    """Fallback (non-default masks): plain numpy replica of the reference."""
    q = np.einsum("bse,hed->bhsd", query, Wq)
    k = np.einsum("bse,hed->bhsd", key, Wk)
    v = np.einsum("bse,hed->bhsd", value, Wv)
    s = np.einsum("bhsd,bhtd->bhst", q, k)
    if decoder_mask:
        tril = np.tril(s)
        s = np.where(tril == 0.0, -np.inf, s)
    s = np.where(padding_mask[:, None, :, :], s, -np.inf)
    s = s / np.sqrt(np.float32(DK_H))
    m = np.max(s, axis=-1, keepdims=True)
    e = np.exp(s - m)
    a = e / np.sum(e, axis=-1, keepdims=True)
    o = np.einsum("bhst,bhtd->bhsd", a, v)
    o = o.transpose(0, 2, 1, 3).reshape(o.shape[0], o.shape[2], H * DV_H)
    return (o @ Wo + bo).astype(np.float32)


def kernel(query, key, value, padding_mask, decoder_mask, Wq, Wk, Wv, Wo, bo,
           **run_kwargs):
    query = np.asarray(query, np.float32)
    key = np.asarray(key, np.float32)
    value = np.asarray(value, np.float32)
    Wq = np.asarray(Wq, np.float32)
    Wk = np.asarray(Wk, np.float32)
    Wv = np.asarray(Wv, np.float32)
    Wo = np.asarray(Wo, np.float32)
    bo = np.asarray(bo, np.float32)
    pm = np.asarray(padding_mask)
    dm = int(np.asarray(decoder_mask))

    if not bool(pm.all()) or not dm:
        return _reference_numpy(
            query, key, value, pm.astype(bool), dm, Wq, Wk, Wv, Wo, bo
        )

    from concourse.bass_utils import run_bass_kernel_spmd

    if "nc" not in _cache:
        _cache["nc"] = _build()
    nc = _cache["nc"]

    in_maps = _prep_inputs(query, key, value, Wq, Wk, Wv, Wo)
    res = run_bass_kernel_spmd(nc, in_maps, list(range(NCORES)), **run_kwargs)

    outp = np.empty((B, S, E), np.float32)
    for b in range(B):
        fT = (res.results[2 * b]["out"].astype(np.float32)
              + res.results[2 * b + 1]["out"].astype(np.float32))
        outp[b] = fT.T + bo
    if run_kwargs:
        kernel.last_result = res
    return outp


# revision 10
# speedup vs baseline: 1.8867x; 1.1462x over previous
"""Bass/Trainium2 kernel for nn_MultiHeadAttention (B=4, S=2048, E=512, H=8, dk=dv=8).

Sharding: 8 cores = (batch b, head-half hh).  Core 2b+hh computes causal
attention for batch b over heads [4hh, 4hh+4) for all 2048 queries, applies
its half of the output projection, and returns a partial output transposed
[E, S] in bf16.  Host sums the two partials per batch, transposes, adds bo.

Device layout notes (v2 — bf16 datapath):
  - Everything the PE touches is bf16 (fp32-HIGH matmuls are ~2.5x slower
    and block fast-weight-load).  PSUM accumulation stays f32.
  - Host feeds query/key/value TRANSPOSED ([E, S]) bf16; q/k/v live whole
    in SBUF (16 KiB/partition each), loaded with per-chunk DMAs.
  - Projection weights host-packed "spread": Q/K heads at partition cols
    {32h..32h+8}; V at cols {9h+1..9h+9} of a 36-wide block (col 9h+0
    feeds the softmax denominator via a ones column); Wo rows at
    {32h+1..32h+9}.
  - Scores are computed transposed ([t, q]): 4 heads concurrently via PE
    row-tiling (tile_position=(32h,0), K=8), one [128,4,512] PSUM tile
    (4 banks), ONE 2048-wide ACT exp per (chunk, t-block) writing bf16.
  - The causal mask of diagonal blocks is ADDED ON THE PE (stationary
    mask^T x identity accumulated into the scores psum) so no vector-op
    sits between matmul and exp.
  - A@V via col-tiling (tile_position=(0,32h)) with a [128 t, 32] bf16
    stationary (8 V dims + ones + 23 zero cols) so the whole [128,512]
    accumulator psum is matmul-written (no uninitialized reads).
  - Softmax normalization folded to the end of each chunk: one full-width
    DVE reciprocal_approx_fast on the accumulator, a rank-1 PE matmul
    replicates each head's denominator row across its 32-partition group,
    one full-width DVE multiply produces the bf16 out-proj operand.
"""

import math

import numpy as np

B, S, E, H = 4, 2048, 512, 8
DK_H = DV_H = 8
NCORES = 8
HPC = H // 2  # heads per core = 4
SCALE = 1.0 / math.sqrt(DK_H)
NEG = -1.0e30
NQC = S // 512  # q chunks of 512
NTB = S // 128  # t blocks of 128
ECH = E // 128  # e chunks of 128

_cache: dict = {}


def _apply_tile_patch():
    """walrus in this image allows only one sync-wait per Drain; split the
    TileContext tail drain's waits across a chain of drains."""
    import concourse.mybir as mybir
    from concourse import tile
    from concourse.vector_clock import ScopedClock

    if getattr(tile.TileContext._drain_and_barrier, "_split_patch", False):
        return

    def _drain_and_barrier_split(self, tick_clock, wait_clock):
        drain_inst = self.nc.sync.drain()
        wait_clock.add_sem_waits(
            drain_inst.ins, ScopedClock({None: tick_clock.global_clock})
        )
        si = drain_inst.ins.sync_info
        if si is not None and si.on_wait and len(si.on_wait) > 1:
            waits = list(si.on_wait)
            si.on_wait = waits[:1]
            for entry in waits[1:]:
                extra = self.nc.sync.drain()
                extra.ins.sync_info = mybir.SyncInfo(on_wait=[entry], on_update=[])
        self.nc.all_engine_barrier()
        assert self.sems is not None
        popped = self.nc._tile_sem_poison_stack.pop()
        assert popped is self._sem_poison
        self.nc.clear_and_free_semaphores(list(self.sems.allocated().values()))
        self.nc.all_engine_barrier()

    _drain_and_barrier_split._split_patch = True
    tile.TileContext._drain_and_barrier = _drain_and_barrier_split


def _split_multi_waits(nc):
    """walrus in this image allows only one sync-wait per instruction;
    move excess waits onto single-wait NOPs inserted just before."""
    import concourse.mybir as mybir

    for blk in nc.m.functions[0].blocks:
        out = []
        for inst in blk.instructions:
            si = getattr(inst, "sync_info", None)
            if si is not None and si.on_wait and len(si.on_wait) > 1:
                waits = list(si.on_wait)
                for i, entry in enumerate(waits[:-1]):
                    out.append(
                        mybir.InstNoOp(
                            name=f"{inst.name}_w{i}",
                            engine=inst.engine,
                            ins=[],
                            outs=[],
                            bass_nofuse=True,
                            sync_info=mybir.SyncInfo(
                                on_wait=[entry], on_update=[]
                            ),
                        )
                    )
                si.on_wait = waits[-1:]
            out.append(inst)
        blk.instructions = out


def _build():
    import concourse.bass as bassmod
    import concourse.mybir as mybir
    from concourse import tile

    _apply_tile_patch()
    f32 = mybir.dt.float32
    bf16 = mybir.dt.bfloat16
    Exp = mybir.ActivationFunctionType.Exp

    nc = bassmod.Bass()
    qT = nc.declare_dram_parameter("qT", [E, S], bf16, isOutput=False)
    kT = nc.declare_dram_parameter("kT", [E, S], bf16, isOutput=False)
    vT = nc.declare_dram_parameter("vT", [E, S], bf16, isOutput=False)
    wq = nc.declare_dram_parameter("wq", [E, 128], bf16, isOutput=False)
    wk = nc.declare_dram_parameter("wk", [E, 128], bf16, isOutput=False)
    wv = nc.declare_dram_parameter("wv", [E, HPC * 9], bf16, isOutput=False)
    wo = nc.declare_dram_parameter("wo", [128, E], bf16, isOutput=False)
    mskT = nc.declare_dram_parameter("mskT", [128, 128], bf16, isOutput=False)
    ident = nc.declare_dram_parameter("ident", [128, 128], bf16, isOutput=False)
    out = nc.declare_dram_parameter("out", [E, S], bf16, isOutput=True)

    with tile.TileContext(nc) as tc:
        with (
            tc.tile_pool(name="singles", bufs=1) as singles,
            tc.tile_pool(name="ats", bufs=4) as atpool,
            tc.tile_pool(name="onorm", bufs=2) as onpool,
            tc.tile_pool(name="outs", bufs=2) as outs,
            tc.tile_pool(name="rc", bufs=2) as rcpool,
            tc.tile_pool(name="ps_sc", bufs=2, space="PSUM") as ps_sc,
            tc.tile_pool(name="ps_av", bufs=2, space="PSUM") as ps_av,
            tc.tile_pool(name="ps_misc", bufs=2, space="PSUM") as ps_misc,
        ):
            # ---- resident tensors -------------------------------------
            wq_sb = singles.tile([128, ECH, 128], bf16, tag="wq")
            wk_sb = singles.tile([128, ECH, 128], bf16, tag="wk")
            wv_sb = singles.tile([128, ECH, HPC * 9], bf16, tag="wv")
            wo_sb = singles.tile([128, ECH, 128], bf16, tag="wo")
            msk_sb = singles.tile([128, 128], bf16, tag="msk")
            id_sb = singles.tile([128, 128], bf16, tag="id")
            nc.sync.dma_start(out=wq_sb, in_=wq.rearrange("(c p) m -> p c m", p=128))
            nc.sync.dma_start(out=wk_sb, in_=wk.rearrange("(c p) m -> p c m", p=128))
            nc.sync.dma_start(out=wv_sb, in_=wv.rearrange("(c p) m -> p c m", p=128))
            nc.sync.dma_start(out=wo_sb, in_=wo.rearrange("p (c m) -> p c m", c=ECH))
            nc.sync.dma_start(out=msk_sb, in_=mskT[:, :])
            nc.sync.dma_start(out=id_sb, in_=ident[:, :])

            q_sb = singles.tile([128, ECH, S], bf16, tag="q")
            k_sb = singles.tile([128, ECH, S], bf16, tag="k")
            v_sb = singles.tile([128, ECH, S], bf16, tag="v")
            kTr = kT.rearrange("(c p) m -> p c m", p=128)
            vTr = vT.rearrange("(c p) m -> p c m", p=128)
            qTr = qT.rearrange("(c p) m -> p c m", p=128)
            for t in range(NQC):
                cs = slice(t * 512, (t + 1) * 512)
                nc.sync.dma_start(out=k_sb[:, :, cs], in_=kTr[:, :, cs])
                nc.sync.dma_start(out=v_sb[:, :, cs], in_=vTr[:, :, cs])
                nc.sync.dma_start(out=q_sb[:, :, cs], in_=qTr[:, :, cs])

            KTs = singles.tile([128, S], bf16, tag="KTs")
            QTs = singles.tile([128, S], bf16, tag="QTs")
            Vsb = singles.tile([128, NTB, HPC, 32], bf16, tag="Vsb")
            ones32 = singles.tile([128, 32], f32, tag="ones32")
            nc.vector.memset(ones32, 1.0)
            # V stationary: col 0 = ones (denominator), 1..8 = V dims,
            # 9..31 = zeros (pads the A@V psum write to the full group).
            nc.vector.memset(Vsb[:, :, :, 0:1], 1.0)
            nc.vector.memset(Vsb[:, :, :, 9:32], 0.0)

            # ---- phase A pieces (projections), interleaved into B -----
            def a_piece_k(t):
                cs = slice(t * 512, (t + 1) * 512)
                kt_ps = ps_misc.tile([128, 512], f32, tag="ps")
                for e in range(ECH):
                    nc.tensor.matmul(
                        kt_ps, wk_sb[:, e, :], k_sb[:, e, cs],
                        start=(e == 0), stop=(e == ECH - 1),
                    )
                nc.vector.tensor_copy(KTs[:, cs], kt_ps)

            def a_piece_q(t):
                cs = slice(t * 512, (t + 1) * 512)
                qt_ps = ps_misc.tile([128, 512], f32, tag="ps")
                for e in range(ECH):
                    nc.tensor.matmul(
                        qt_ps, wq_sb[:, e, :], q_sb[:, e, cs],
                        start=(e == 0), stop=(e == ECH - 1),
                    )
                nc.vector.tensor_copy(QTs[:, cs], qt_ps)

            def a_piece_v(t, tb):
                bs = slice(t * 512 + tb * 128, t * 512 + (tb + 1) * 128)
                v_ps = ps_misc.tile([128, HPC * 9], f32, tag="ps")
                for e in range(ECH):
                    nc.tensor.matmul(
                        v_ps,
                        v_sb[:, e, bs],
                        wv_sb[:, e, :],
                        start=(e == 0), stop=(e == ECH - 1),
                    )
                dst = Vsb[:, 4 * t + tb, :, 1:9]
                src = v_ps.rearrange("p (h n) -> p h n", n=9)[:, :, 1:9]
                nc.vector.tensor_copy(dst, src)

            def a_chunk_pieces(t):
                return [lambda t=t: a_piece_k(t)] + [
                    (lambda t=t, tb=tb: a_piece_v(t, tb)) for tb in range(4)
                ] + [lambda t=t: a_piece_q(t)]

            for piece in a_chunk_pieces(0):
                piece()

            # ---- phase B: attention, software-pipelined ---------------
            # Half-units j = (chunk c, t-block tb, head-group g).  Score
            # psum tiles are 2 banks x 2 buffers so sc(j+2) computes on
            # the PE while ACT exps sc(j); sc(j+2) is emitted BEFORE
            # av(j) so the PE never queues behind the exp it feeds.
            outr = out.rearrange("(c p) m -> p c m", p=128)
            halves = []
            for c in range(NQC):
                for tb in range(4 * (c + 1)):
                    for g in range(2):
                        halves.append((c, tb, g))
            nj = len(halves)

            av_tiles = {}
            sc_tiles = {}
            at_tiles = {}
            # phase-A pieces for chunk t are spread across chunk t-1's units
            pending_a = {c: a_chunk_pieces(c) for c in range(1, NQC)}

            def emit_sc(j):
                c, tb, g = halves[j]
                cs = slice(c * 512, (c + 1) * 512)
                d = 128 * tb - 512 * c
                sc = ps_sc.tile([128, 2, 512], f32, tag="sc",
                                name=f"sc{c}_{tb}_{g}")
                sc_tiles[j] = sc
                for i in range(2):
                    h = 2 * g + i
                    nc.tensor.matmul(
                        sc[:, i, :],
                        KTs[32 * h:32 * h + 8, tb * 128:(tb + 1) * 128],
                        QTs[32 * h:32 * h + 8, cs],
                        start=True, stop=(d < 0),
                        tile_position=(32 * h, 0),
                    )
                if d >= 0:
                    for i in range(2):
                        nc.tensor.matmul(
                            sc[:, i, d:d + 128],
                            msk_sb,
                            id_sb,
                            start=False, stop=True,
                        )

            def emit_exp(j):
                c, tb, g = halves[j]
                d = 128 * tb - 512 * c
                vstart = max(d, 0)
                at = atpool.tile([128, 2, 512], bf16, tag="a",
                                 name=f"a{c}_{tb}_{g}")
                at_tiles[j] = at
                sc = sc_tiles[j]
                nc.scalar.activation(
                    at[:, :, vstart:512], sc[:, :, vstart:512],
                    Exp, scale=SCALE,
                )

            def emit_av(j):
                c, tb, g = halves[j]
                d = 128 * tb - 512 * c
                vstart = max(d, 0)
                ntb = 4 * (c + 1)
                if c not in av_tiles:
                    av_tiles[c] = ps_av.tile([128, 512], f32, tag="av",
                                             name=f"av{c}")
                av = av_tiles[c]
                at = at_tiles.pop(j)
                for i in range(2):
                    h = 2 * g + i
                    nc.tensor.matmul(
                        av[32 * h:32 * h + 32, vstart:512],
                        Vsb[:, tb, h, :],
                        at[:, i, vstart:512],
                        start=(tb == 0), stop=(tb == ntb - 1),
                        tile_position=(0, 32 * h),
                    )
                del sc_tiles[j]

            def emit_norm(c):
                # normalize: full-width reciprocal of the accumulator (the
                # denominator rows among it), replicate each head's denom
                # row across its 32-partition group via a K=1 rank-1
                # matmul, then one full-width multiply -> bf16 operand.
                cs = slice(c * 512, (c + 1) * 512)
                av = av_tiles.pop(c)
                recipf = rcpool.tile([128, 512], f32, tag="rc", name=f"rc{c}")
                nc.vector.reciprocal(recipf, av)
                rep_ps = ps_misc.tile([128, 512], f32, tag="ps",
                                      name=f"rep{c}")
                for h in range(HPC):
                    nc.tensor.matmul(
                        rep_ps[32 * h:32 * h + 32, :],
                        ones32[32 * h:32 * h + 1, :],
                        recipf[32 * h:32 * h + 1, :],
                        start=True, stop=True,
                        tile_position=(32 * h, 32 * h),
                    )
                rep_sb = rcpool.tile([128, 512], f32, tag="rs", name=f"rs{c}")
                nc.vector.tensor_copy(rep_sb, rep_ps)
                onorm = onpool.tile([128, 512], bf16, tag="on", name=f"on{c}")
                nc.vector.tensor_mul(onorm, av, rep_sb)
                fsb = outs.tile([128, ECH, 512], bf16, tag="f", name=f"f{c}")
                for e in range(ECH):
                    f_ps = ps_misc.tile([128, 512], f32, tag="ps")
                    nc.tensor.matmul(
                        f_ps, wo_sb[:, e, :], onorm,
                        start=True, stop=True,
                    )
                    nc.vector.tensor_copy(fsb[:, e, :], f_ps)
                nc.sync.dma_start(out=outr[:, :, cs], in_=fsb)

            def ensure_pieces(cc):
                # all of chunk cc's projections must be EMITTED before any
                # of chunk cc's score units (Tile synchronizes program
                # order; a reader emitted before its writer reads stale
                # data).
                while pending_a.get(cc):
                    pending_a[cc].pop(0)()

            emit_sc(0)
            emit_sc(1)
            for j in range(nj):
                c, tb, g = halves[j]
                emit_exp(j)
                if j + 2 < nj:
                    ensure_pieces(halves[j + 2][0])
                    emit_sc(j + 2)
                emit_av(j)
                # interleave next chunk's projections (one piece per
                # half-unit) and flush chunk-end normalization
                if pending_a.get(c + 1):
                    pending_a[c + 1].pop(0)()
                if tb == 4 * (c + 1) - 1 and g == 1:
                    emit_norm(c)
    _split_multi_waits(nc)
    return nc


def _prep_inputs(query, key, value, Wq, Wk, Wv, Wo):
    """Build the 8 per-core input maps (host-side sharding/layout)."""
    import ml_dtypes

    bf = ml_dtypes.bfloat16
    qTs = [np.ascontiguousarray(query[b].T).astype(bf) for b in range(B)]
    kTs = [np.ascontiguousarray(key[b].T).astype(bf) for b in range(B)]
    vTs = [np.ascontiguousarray(value[b].T).astype(bf) for b in range(B)]

    # mskT[k, m] = 0 if m <= k else NEG  (== msk.T with msk[t, q] causal)
    mskT = np.where(
        np.arange(128)[None, :] <= np.arange(128)[:, None], 0.0, NEG
    ).astype(np.float32).astype(bf)
    mskT = np.ascontiguousarray(mskT)
    ident = np.eye(128, dtype=np.float32).astype(bf)

    in_maps = []
    for core in range(NCORES):
        b, hh = divmod(core, 2)
        wq_p = np.zeros((E, 128), np.float32)
        wk_p = np.zeros((E, 128), np.float32)
        wv_p = np.zeros((E, HPC * 9), np.float32)
        wo_p = np.zeros((128, E), np.float32)
        for h in range(HPC):
            g = 4 * hh + h
            wq_p[:, 32 * h:32 * h + 8] = Wq[g]
            wk_p[:, 32 * h:32 * h + 8] = Wk[g]
            wv_p[:, 9 * h + 1:9 * h + 9] = Wv[g]
            wo_p[32 * h + 1:32 * h + 9, :] = Wo[8 * g:8 * g + 8, :]
        in_maps.append(
            {
                "qT": qTs[b], "kT": kTs[b], "vT": vTs[b],
                "wq": wq_p.astype(bf), "wk": wk_p.astype(bf),
                "wv": wv_p.astype(bf), "wo": wo_p.astype(bf),
                "mskT": mskT, "ident": ident,
            }
        )
    return in_maps


def _reference_numpy(query, key, value, padding_mask, decoder_mask,
                     Wq, Wk, Wv, Wo, bo):
    """Fallback (non-default masks): plain numpy replica of the reference."""
    q = np.einsum("bse,hed->bhsd", query, Wq)
    k = np.einsum("bse,hed->bhsd", key, Wk)
    v = np.einsum("bse,hed->bhsd", value, Wv)
    s = np.einsum("bhsd,bhtd->bhst", q, k)
    if decoder_mask:
        tril = np.tril(s)
        s = np.where(tril == 0.0, -np.inf, s)
    s = np.where(padding_mask[:, None, :, :], s, -np.inf)
    s = s / np.sqrt(np.float32(DK_H))
    m = np.max(s, axis=-1, keepdims=True)
    e = np.exp(s - m)
    a = e / np.sum(e, axis=-1, keepdims=True)
    o = np.einsum("bhst,bhtd->bhsd", a, v)
    o = o.transpose(0, 2, 1, 3).reshape(o.shape[0], o.shape[2], H * DV_H)
    return (o @ Wo + bo).astype(np.float32)


def kernel(query, key, value, padding_mask, decoder_mask, Wq, Wk, Wv, Wo, bo,
           **run_kwargs):
    query = np.asarray(query, np.float32)
    key = np.asarray(key, np.float32)
    value = np.asarray(value, np.float32)
    Wq = np.asarray(Wq, np.float32)
    Wk = np.asarray(Wk, np.float32)
    Wv = np.asarray(Wv, np.float32)
    Wo = np.asarray(Wo, np.float32)
    bo = np.asarray(bo, np.float32)
    pm = np.asarray(padding_mask)
    dm = int(np.asarray(decoder_mask))

    if not bool(pm.all()) or not dm:
        return _reference_numpy(
            query, key, value, pm.astype(bool), dm, Wq, Wk, Wv, Wo, bo
        )

    from concourse.bass_utils import run_bass_kernel_spmd

    if "nc" not in _cache:
        _cache["nc"] = _build()
    nc = _cache["nc"]

    in_maps = _prep_inputs(query, key, value, Wq, Wk, Wv, Wo)
    res = run_bass_kernel_spmd(nc, in_maps, list(range(NCORES)), **run_kwargs)

    outp = np.empty((B, S, E), np.float32)
    for b in range(B):
        fT = (res.results[2 * b]["out"].astype(np.float32)
              + res.results[2 * b + 1]["out"].astype(np.float32))
        outp[b] = fT.T + bo
    if run_kwargs:
        kernel.last_result = res
    return outp


# revision 18
# speedup vs baseline: 2.0541x; 1.0888x over previous
"""Bass/Trainium2 kernel for nn_MultiHeadAttention (B=4, S=2048, E=512, H=8, dk=dv=8).

Sharding: 8 cores = (batch b, head-half hh).  Core 2b+hh computes causal
attention for batch b over heads [4hh, 4hh+4) for all 2048 queries, applies
its half of the output projection, and returns a partial output transposed
[E, S] in bf16.  Host sums the two partials per batch, transposes, adds bo.

Device layout notes (v2 — bf16 datapath):
  - Everything the PE touches is bf16 (fp32-HIGH matmuls are ~2.5x slower
    and block fast-weight-load).  PSUM accumulation stays f32.
  - Host feeds query/key/value TRANSPOSED ([E, S]) bf16; q/k/v live whole
    in SBUF (16 KiB/partition each), loaded with per-chunk DMAs.
  - Projection weights host-packed "spread": Q/K heads at partition cols
    {32h..32h+8}; V at cols {9h+1..9h+9} of a 36-wide block (col 9h+0
    feeds the softmax denominator via a ones column); Wo rows at
    {32h+1..32h+9}.
  - Scores are computed transposed ([t, q]): 4 heads concurrently via PE
    row-tiling (tile_position=(32h,0), K=8), one [128,4,512] PSUM tile
    (4 banks), ONE 2048-wide ACT exp per (chunk, t-block) writing bf16.
  - The causal mask of diagonal blocks is ADDED ON THE PE (stationary
    mask^T x identity accumulated into the scores psum) so no vector-op
    sits between matmul and exp.
  - A@V via col-tiling (tile_position=(0,32h)) with a [128 t, 32] bf16
    stationary (8 V dims + ones + 23 zero cols) so the whole [128,512]
    accumulator psum is matmul-written (no uninitialized reads).
  - Softmax normalization folded to the end of each chunk: one full-width
    DVE reciprocal_approx_fast on the accumulator, a rank-1 PE matmul
    replicates each head's denominator row across its 32-partition group,
    one full-width DVE multiply produces the bf16 out-proj operand.
"""

import math

import numpy as np

B, S, E, H = 4, 2048, 512, 8
DK_H = DV_H = 8
NCORES = 8
HPC = H // 2  # heads per core = 4
SCALE = 1.0 / math.sqrt(DK_H)
NEG = -1.0e30
NQC = S // 512  # q chunks of 512
NTB = S // 128  # t blocks of 128
ECH = E // 128  # e chunks of 128

_cache: dict = {}


def _apply_tile_patch():
    """walrus in this image allows only one sync-wait per Drain; split the
    TileContext tail drain's waits across a chain of drains."""
    import concourse.mybir as mybir
    from concourse import tile
    from concourse.vector_clock import ScopedClock

    if getattr(tile.TileContext._drain_and_barrier, "_split_patch", False):
        return

    def _drain_and_barrier_split(self, tick_clock, wait_clock):
        drain_inst = self.nc.sync.drain()
        wait_clock.add_sem_waits(
            drain_inst.ins, ScopedClock({None: tick_clock.global_clock})
        )
        si = drain_inst.ins.sync_info
        if si is not None and si.on_wait and len(si.on_wait) > 1:
            waits = list(si.on_wait)
            si.on_wait = waits[:1]
            for entry in waits[1:]:
                extra = self.nc.sync.drain()
                extra.ins.sync_info = mybir.SyncInfo(on_wait=[entry], on_update=[])
        self.nc.all_engine_barrier()
        assert self.sems is not None
        popped = self.nc._tile_sem_poison_stack.pop()
        assert popped is self._sem_poison
        self.nc.clear_and_free_semaphores(list(self.sems.allocated().values()))
        self.nc.all_engine_barrier()

    _drain_and_barrier_split._split_patch = True
    tile.TileContext._drain_and_barrier = _drain_and_barrier_split


def _split_multi_waits(nc):
    """walrus in this image allows only one sync-wait per instruction;
    move excess waits onto single-wait NOPs inserted just before."""
    import concourse.mybir as mybir

    for blk in nc.m.functions[0].blocks:
        out = []
        for inst in blk.instructions:
            si = getattr(inst, "sync_info", None)
            if si is not None and si.on_wait and len(si.on_wait) > 1:
                waits = list(si.on_wait)
                for i, entry in enumerate(waits[:-1]):
                    out.append(
                        mybir.InstNoOp(
                            name=f"{inst.name}_w{i}",
                            engine=inst.engine,
                            ins=[],
                            outs=[],
                            bass_nofuse=True,
                            sync_info=mybir.SyncInfo(
                                on_wait=[entry], on_update=[]
                            ),
                        )
                    )
                si.on_wait = waits[-1:]
            out.append(inst)
        blk.instructions = out


def _build():
    import concourse.bass as bassmod
    import concourse.mybir as mybir
    from concourse import tile

    _apply_tile_patch()
    f32 = mybir.dt.float32
    bf16 = mybir.dt.bfloat16
    Exp = mybir.ActivationFunctionType.Exp

    nc = bassmod.Bass()
    qT = nc.declare_dram_parameter("qT", [E, S], bf16, isOutput=False)
    kT = nc.declare_dram_parameter("kT", [E, S], bf16, isOutput=False)
    vT = nc.declare_dram_parameter("vT", [E, S], bf16, isOutput=False)
    wq = nc.declare_dram_parameter("wq", [E, 128], bf16, isOutput=False)
    wk = nc.declare_dram_parameter("wk", [E, 128], bf16, isOutput=False)
    wv = nc.declare_dram_parameter("wv", [E, HPC * 9], bf16, isOutput=False)
    wo = nc.declare_dram_parameter("wo", [128, E], bf16, isOutput=False)
    mskT = nc.declare_dram_parameter("mskT", [128, 128], bf16, isOutput=False)
    ident = nc.declare_dram_parameter("ident", [128, 128], bf16, isOutput=False)
    out = nc.declare_dram_parameter("out", [E, S], bf16, isOutput=True)

    with tile.TileContext(nc) as tc:
        with (
            tc.tile_pool(name="singles", bufs=1) as singles,
            tc.tile_pool(name="ats", bufs=4) as atpool,
            tc.tile_pool(name="onorm", bufs=2) as onpool,
            tc.tile_pool(name="outs", bufs=2) as outs,
            tc.tile_pool(name="rc", bufs=2) as rcpool,
            tc.tile_pool(name="ps_sc", bufs=2, space="PSUM") as ps_sc,
            tc.tile_pool(name="ps_av", bufs=2, space="PSUM") as ps_av,
            tc.tile_pool(name="ps_misc", bufs=2, space="PSUM") as ps_misc,
        ):
            # ---- resident tensors -------------------------------------
            wq_sb = singles.tile([128, ECH, 128], bf16, tag="wq")
            wk_sb = singles.tile([128, ECH, 128], bf16, tag="wk")
            wv_sb = singles.tile([128, ECH, HPC * 9], bf16, tag="wv")
            wo_sb = singles.tile([128, ECH, 128], bf16, tag="wo")
            msk_sb = singles.tile([128, 128], bf16, tag="msk")
            id_sb = singles.tile([128, 128], bf16, tag="id")

            q_sb = singles.tile([128, ECH, S], bf16, tag="q")
            k_sb = singles.tile([128, ECH, S], bf16, tag="k")
            v_sb = singles.tile([128, ECH, S], bf16, tag="v")
            kTr = kT.rearrange("(c p) m -> p c m", p=128)
            vTr = vT.rearrange("(c p) m -> p c m", p=128)
            qTr = qT.rearrange("(c p) m -> p c m", p=128)

            # DMA order: what chunk 0 of the compute needs comes first so
            # the pipeline starts ~11us earlier.
            nc.sync.dma_start(out=wk_sb, in_=wk.rearrange("(c p) m -> p c m", p=128))
            nc.sync.dma_start(out=wq_sb, in_=wq.rearrange("(c p) m -> p c m", p=128))
            cs0 = slice(0, 512)
            nc.sync.dma_start(out=k_sb[:, :, cs0], in_=kTr[:, :, cs0])
            nc.sync.dma_start(out=q_sb[:, :, cs0], in_=qTr[:, :, cs0])
            nc.sync.dma_start(out=wv_sb, in_=wv.rearrange("(c p) m -> p c m", p=128))
            nc.sync.dma_start(out=v_sb[:, :, cs0], in_=vTr[:, :, cs0])
            nc.sync.dma_start(out=msk_sb, in_=mskT[:, :])
            nc.sync.dma_start(out=wo_sb, in_=wo.rearrange("p (c m) -> p c m", c=ECH))
            nc.sync.dma_start(out=id_sb, in_=ident[:, :])
            for t in range(1, NQC):
                cs = slice(t * 512, (t + 1) * 512)
                nc.sync.dma_start(out=k_sb[:, :, cs], in_=kTr[:, :, cs])
                nc.sync.dma_start(out=q_sb[:, :, cs], in_=qTr[:, :, cs])
                nc.sync.dma_start(out=v_sb[:, :, cs], in_=vTr[:, :, cs])

            KTs = singles.tile([128, S], bf16, tag="KTs")
            QTs = singles.tile([128, S], bf16, tag="QTs")
            Vsb = singles.tile([128, NTB, HPC, 32], bf16, tag="Vsb")
            ones32 = singles.tile([128, 32], bf16, tag="ones32")
            nc.vector.memset(ones32, 1.0)
            # warm the ACT exp table while DMAs run (table load ~2.7us)
            warm_sb = singles.tile([128, 8], f32, tag="warm")
            warm_out = singles.tile([128, 8], f32, tag="warmo")
            nc.vector.memset(warm_sb, 0.0)
            nc.scalar.activation(warm_out, warm_sb, Exp, scale=1.0)
            # V stationary: col 0 = ones (denominator), 1..8 = V dims,
            # 9..31 = zeros (pads the A@V psum write to the full group).
            nc.vector.memset(Vsb[:, :, :, 0:1], 1.0)
            nc.vector.memset(Vsb[:, :, :, 9:32], 0.0)

            # ---- phase A pieces (projections), interleaved into B -----
            def a_piece_k(t):
                cs = slice(t * 512, (t + 1) * 512)
                kt_ps = ps_misc.tile([128, 512], f32, tag="ps")
                for e in range(ECH):
                    nc.tensor.matmul(
                        kt_ps, wk_sb[:, e, :], k_sb[:, e, cs],
                        start=(e == 0), stop=(e == ECH - 1),
                    )
                nc.vector.tensor_copy(KTs[:, cs], kt_ps)

            def a_piece_q(t):
                cs = slice(t * 512, (t + 1) * 512)
                qt_ps = ps_misc.tile([128, 512], f32, tag="ps")
                for e in range(ECH):
                    nc.tensor.matmul(
                        qt_ps, wq_sb[:, e, :], q_sb[:, e, cs],
                        start=(e == 0), stop=(e == ECH - 1),
                    )
                nc.vector.tensor_copy(QTs[:, cs], qt_ps)

            def a_piece_v(t, tb):
                bs = slice(t * 512 + tb * 128, t * 512 + (tb + 1) * 128)
                v_ps = ps_misc.tile([128, HPC * 9], f32, tag="ps")
                for e in range(ECH):
                    nc.tensor.matmul(
                        v_ps,
                        v_sb[:, e, bs],
                        wv_sb[:, e, :],
                        start=(e == 0), stop=(e == ECH - 1),
                    )
                dst = Vsb[:, 4 * t + tb, :, 1:9]
                src = v_ps.rearrange("p (h n) -> p h n", n=9)[:, :, 1:9]
                nc.vector.tensor_copy(dst, src)

            def a_chunk_pieces(t):
                return [lambda t=t: a_piece_k(t),
                        lambda t=t: a_piece_q(t)] + [
                    (lambda t=t, tb=tb: a_piece_v(t, tb)) for tb in range(4)
                ]

            for piece in a_chunk_pieces(0):
                piece()

            # ---- phase B: attention, software-pipelined ---------------
            # Half-units j = (chunk c of 512 queries, t-block tb, head
            # group g of 2): 2-way row-tiled scores per group into a
            # 2-bank psum tile (1 bank per head plane -- concurrent PE
            # drains to one partition-bank are a HW conflict), 2 buffers
            # so sc(j+2) computes on the PE while ACT exps sc(j);
            # sc(j+2) is emitted BEFORE av(j) so the PE never queues
            # behind the exp it feeds.
            outr = out.rearrange("(c p) m -> p c m", p=128)
            units = []
            for c in range(NQC):
                for tb in range(4 * (c + 1)):
                    for g in range(2):
                        units.append((c, tb, g))
            nj = len(units)

            av_tiles = {}
            sc_tiles = {}
            at_tiles = {}
            # projections for chunk T must be emitted before chunk T's
            # score units; drip one piece per half-unit to spread PE load
            a_queue = []
            for T in range(1, NQC):
                for p in a_chunk_pieces(T):
                    a_queue.append((T, p))

            def drain_a(c_needed=None, drip=False):
                while a_queue and (
                    (c_needed is not None and a_queue[0][0] <= c_needed)
                    or drip
                ):
                    a_queue.pop(0)[1]()
                    drip = False

            def emit_sc(j):
                c, tb, g = units[j]
                d = 128 * tb - 512 * c
                vstart = max(d, 0)
                sc = ps_sc.tile([128, 2, 512], f32, tag="sc",
                                name=f"sc{c}_{tb}_{g}")
                sc_tiles[j] = sc
                for i in range(2):
                    h = 2 * g + i
                    nc.tensor.matmul(
                        sc[:, i, vstart:512],
                        KTs[32 * h:32 * h + 8, tb * 128:(tb + 1) * 128],
                        QTs[32 * h:32 * h + 8,
                            c * 512 + vstart:(c + 1) * 512],
                        start=True, stop=(d < 0),
                        tile_position=(32 * h, 0),
                    )
                if d >= 0:
                    # causal mask of the diagonal 128-col region, added
                    # on the PE: msk^T (stationary) @ I (moving).
                    for i in range(2):
                        nc.tensor.matmul(
                            sc[:, i, d:d + 128],
                            msk_sb,
                            id_sb,
                            start=False, stop=True,
                        )

            def emit_exp(j):
                c, tb, g = units[j]
                vstart = max(128 * tb - 512 * c, 0)
                at = atpool.tile([128, 2, 512], bf16, tag="a",
                                 name=f"a{c}_{tb}_{g}")
                at_tiles[j] = at
                sc = sc_tiles[j]
                nc.scalar.activation(
                    at[:, :, vstart:512], sc[:, :, vstart:512],
                    Exp, scale=SCALE,
                )

            def emit_av(j):
                c, tb, g = units[j]
                vstart = max(128 * tb - 512 * c, 0)
                ntb = 4 * (c + 1)
                if c not in av_tiles:
                    av_tiles[c] = ps_av.tile([128, 512], f32, tag="av",
                                             name=f"av{c}")
                av = av_tiles[c]
                at = at_tiles.pop(j)
                for i in range(2):
                    h = 2 * g + i
                    nc.tensor.matmul(
                        av[32 * h:32 * h + 32, vstart:512],
                        Vsb[:, tb, h, :],
                        at[:, i, vstart:512],
                        start=(tb == 0), stop=(tb == ntb - 1),
                        tile_position=(0, 32 * h),
                    )
                del sc_tiles[j]

            def emit_norm(c):
                # normalize: full-width reciprocal of the accumulator (the
                # denominator rows among it) -> bf16, replicate each
                # head's denom row across its 32-partition group via a
                # K=1 rank-1 bf16 matmul, then one multiply -> bf16.
                cs = slice(c * 512, (c + 1) * 512)
                av = av_tiles.pop(c)
                recipb = rcpool.tile([128, 512], bf16, tag="rc",
                                     name=f"rc{c}")
                with nc.allow_low_precision(
                    reason="bf16 denominators keep the replicate matmul "
                           "off the slow fp32 PE path; 0.4% rel in budget"
                ):
                    nc.vector.reciprocal(recipb, av)
                rep_ps = ps_misc.tile([128, 512], f32, tag="ps",
                                      name=f"rep{c}")
                for h in range(HPC):
                    nc.tensor.matmul(
                        rep_ps[32 * h:32 * h + 32, :],
                        ones32[32 * h:32 * h + 1, :],
                        recipb[32 * h:32 * h + 1, :],
                        start=True, stop=True,
                        tile_position=(32 * h, 32 * h),
                    )
                rep_sb = rcpool.tile([128, 512], f32, tag="rs",
                                     name=f"rs{c}")
                nc.vector.tensor_copy(rep_sb, rep_ps)
                onorm = onpool.tile([128, 512], bf16, tag="on",
                                    name=f"on{c}")
                nc.vector.tensor_mul(onorm, av, rep_sb)
                fsb = outs.tile([128, ECH, 512], bf16, tag="f",
                                name=f"f{c}")
                for e in range(ECH):
                    f_ps = ps_misc.tile([128, 512], f32, tag="ps",
                                        name=f"fp{c}_{e}")
                    nc.tensor.matmul(
                        f_ps, wo_sb[:, e, :], onorm,
                        start=True, stop=True,
                    )
                    nc.vector.tensor_copy(fsb[:, e, :], f_ps)
                nc.sync.dma_start(out=outr[:, :, cs], in_=fsb)

            emit_sc(0)
            emit_sc(1)
            for j in range(nj):
                c, tb, g = units[j]
                emit_exp(j)
                if j + 2 < nj:
                    drain_a(c_needed=units[j + 2][0])
                    emit_sc(j + 2)
                emit_av(j)
                drain_a(drip=True)
                if tb == 4 * (c + 1) - 1 and g == 1:
                    emit_norm(c)
    _split_multi_waits(nc)
    return nc


def _prep_inputs(query, key, value, Wq, Wk, Wv, Wo):
    """Build the 8 per-core input maps (host-side sharding/layout)."""
    import ml_dtypes

    bf = ml_dtypes.bfloat16
    qTs = [np.ascontiguousarray(query[b].T).astype(bf) for b in range(B)]
    kTs = [np.ascontiguousarray(key[b].T).astype(bf) for b in range(B)]
    vTs = [np.ascontiguousarray(value[b].T).astype(bf) for b in range(B)]

    # mskT[k, m] = 0 if m <= k else NEG  (== msk.T with msk[t, q] causal)
    mskT = np.where(
        np.arange(128)[None, :] <= np.arange(128)[:, None], 0.0, NEG
    ).astype(np.float32).astype(bf)
    mskT = np.ascontiguousarray(mskT)
    ident = np.eye(128, dtype=np.float32).astype(bf)

    in_maps = []
    for core in range(NCORES):
        b, hh = divmod(core, 2)
        wq_p = np.zeros((E, 128), np.float32)
        wk_p = np.zeros((E, 128), np.float32)
        wv_p = np.zeros((E, HPC * 9), np.float32)
        wo_p = np.zeros((128, E), np.float32)
        for h in range(HPC):
            g = 4 * hh + h
            wq_p[:, 32 * h:32 * h + 8] = Wq[g]
            wk_p[:, 32 * h:32 * h + 8] = Wk[g]
            wv_p[:, 9 * h + 1:9 * h + 9] = Wv[g]
            wo_p[32 * h + 1:32 * h + 9, :] = Wo[8 * g:8 * g + 8, :]
        in_maps.append(
            {
                "qT": qTs[b], "kT": kTs[b], "vT": vTs[b],
                "wq": wq_p.astype(bf), "wk": wk_p.astype(bf),
                "wv": wv_p.astype(bf), "wo": wo_p.astype(bf),
                "mskT": mskT, "ident": ident,
            }
        )
    return in_maps


def _reference_numpy(query, key, value, padding_mask, decoder_mask,
                     Wq, Wk, Wv, Wo, bo):
    """Fallback (non-default masks): plain numpy replica of the reference."""
    q = np.einsum("bse,hed->bhsd", query, Wq)
    k = np.einsum("bse,hed->bhsd", key, Wk)
    v = np.einsum("bse,hed->bhsd", value, Wv)
    s = np.einsum("bhsd,bhtd->bhst", q, k)
    if decoder_mask:
        tril = np.tril(s)
        s = np.where(tril == 0.0, -np.inf, s)
    s = np.where(padding_mask[:, None, :, :], s, -np.inf)
    s = s / np.sqrt(np.float32(DK_H))
    m = np.max(s, axis=-1, keepdims=True)
    e = np.exp(s - m)
    a = e / np.sum(e, axis=-1, keepdims=True)
    o = np.einsum("bhst,bhtd->bhsd", a, v)
    o = o.transpose(0, 2, 1, 3).reshape(o.shape[0], o.shape[2], H * DV_H)
    return (o @ Wo + bo).astype(np.float32)


def kernel(query, key, value, padding_mask, decoder_mask, Wq, Wk, Wv, Wo, bo,
           **run_kwargs):
    query = np.asarray(query, np.float32)
    key = np.asarray(key, np.float32)
    value = np.asarray(value, np.float32)
    Wq = np.asarray(Wq, np.float32)
    Wk = np.asarray(Wk, np.float32)
    Wv = np.asarray(Wv, np.float32)
    Wo = np.asarray(Wo, np.float32)
    bo = np.asarray(bo, np.float32)
    pm = np.asarray(padding_mask)
    dm = int(np.asarray(decoder_mask))

    if not bool(pm.all()) or not dm:
        return _reference_numpy(
            query, key, value, pm.astype(bool), dm, Wq, Wk, Wv, Wo, bo
        )

    from concourse.bass_utils import run_bass_kernel_spmd

    if "nc" not in _cache:
        _cache["nc"] = _build()
    nc = _cache["nc"]

    in_maps = _prep_inputs(query, key, value, Wq, Wk, Wv, Wo)
    res = run_bass_kernel_spmd(nc, in_maps, list(range(NCORES)), **run_kwargs)

    outp = np.empty((B, S, E), np.float32)
    for b in range(B):
        fT = (res.results[2 * b]["out"].astype(np.float32)
              + res.results[2 * b + 1]["out"].astype(np.float32))
        outp[b] = fT.T + bo
    if run_kwargs:
        kernel.last_result = res
    return outp
